# revision 51
# baseline (speedup 1.0000x reference)
"""GatedDeltaNet (B=2, T=1024, D=512, H=1) fully on-device on 8 trn2 cores.

Sharding: core (b, s) = batch b x Dv-slice s (128 v-columns of the state).
The sequential delta-rule scan parallelizes over Dv with no cross-core
traffic; the final cross-slice combine is a small AllToAll of the raw
pre-norm o slices (256KB/core) instead of a ReduceScatter of projected
partials (1MB/core); each core then computes RMSNorm + gate + output
projection for its own 256-row t-quarter.

Single launch per 8 cores:
  * bf16 projections (q,k full + v slice) via PE, causal dwconv as
    diagonal-matmuls on PE, silu via the HW act table.
  * beta/a projected directly in column space ([t-part, chunk] layout)
    with 2-col RHS matmuls; in-chunk cumsum of g as a lower-triangular
    ones matmul; l2norm scales via Sqrt+reciprocal (no fp32 1-col
    transpose matmuls, minimal act-table switching).
  * chunked (C=128) gated delta rule with the exact log-depth Neumann
    factorization truncated at (I-M)(I+M^2); bf16 operands, f32 PSUM.
  * o chunks stream to DRAM during the scan; one 4-core AllToAll hands
    each core the full-Dh o rows for its t-quarter; tail computes
    RMSNorm, silu(gate) and the output projection on-device.
HW exec time is the genuine NTFF-profiled device execution time
(max traced core), captured via the axon NRT profile hook.
"""

import sys
import types
import time
from contextlib import ExitStack

import numpy as np
import ml_dtypes

BF16 = ml_dtypes.bfloat16

B, T, D, KC = 2, 1024, 512, 4
P = 128
C = 128          # chunk length
NCH = T // C     # 8 chunks
NDT = D // P     # 4 dk tiles

_LAST_HW_NS = [None]
USE_SILU_TABLE = True   # real HW has a silu table; CoreSim does not


# ─────────────────────────── axon NTFF hook shim ──────────────────────────
def install_ntff_shim():
    if "antenv.axon_hooks" in sys.modules:
        return
    try:
        import antenv  # noqa: F401
        from trn_agent_boot.trn_boot import _ntff_profile_via_ctypes
        hook = _ntff_profile_via_ctypes('/opt/axon/libaxon_pjrt.so')
    except Exception:
        hook = None
    mod = types.ModuleType("antenv.axon_hooks")
    mod.get_axon_ntff_profile_hook = lambda: hook
    mod.set_axon_ntff_profile_hook = lambda h: None
    sys.modules["antenv.axon_hooks"] = mod


# ─────────────────────────────── L1 kernel ────────────────────────────────
def l1_kernel(tc, ins, outs):
    import concourse.bass as bass
    import concourse.mybir as mybir
    from concourse.masks import make_identity

    nc = tc.nc
    fp32 = mybir.dt.float32
    bf16 = mybir.dt.bfloat16
    AF = mybir.ActivationFunctionType
    OP = mybir.AluOpType

    xT, wqk, wv, wba, convw, sc = (ins[k] for k in
                                   ("xT", "wqk", "wv", "wba", "convw", "sc"))
    wg, wo, wnr, xq, bmask = (ins[k] for k in ("wg", "wo", "wnr", "xq", "bmask"))
    out_rows = outs["out"]

    ctx = ExitStack()
    with ctx:
        sing = ctx.enter_context(tc.tile_pool(name="sing", bufs=1))
        sb2 = ctx.enter_context(tc.tile_pool(name="sb2", bufs=2))
        ps = ctx.enter_context(tc.tile_pool(name="ps", bufs=1, space="PSUM"))
        ps2 = ctx.enter_context(tc.tile_pool(name="ps2", bufs=2, space="PSUM"))
        dramp = ctx.enter_context(tc.tile_pool(name="dramp", bufs=1, space="DRAM"))

        # ── load inputs (split big tensors for DMA/compute overlap) ──
        TP = 1027  # padded t-block length
        xT_sb = sing.tile([P, NDT * TP], bf16)
        xT_v = xT.rearrange("p (k t) -> p k t", k=NDT)
        xTs_v = xT_sb.rearrange("p (k t) -> p k t", k=NDT)
        for kt, eng_ in ((0, nc.sync), (1, nc.scalar), (2, nc.gpsimd),
                         (3, nc.sync)):
            eng_.dma_start(xTs_v[:, kt], xT_v[:, kt])
        wqk_sb = sing.tile([P, NDT * 1024], bf16)
        wqk_v = wqk.rearrange("p (k t) -> p k t", k=NDT)
        wqks_v = wqk_sb.rearrange("p (k t) -> p k t", k=NDT)
        for th, eng_ in ((0, nc.gpsimd), (1, nc.scalar)):
            eng_.dma_start(wqks_v[:, :, th * 512:(th + 1) * 512],
                           wqk_v[:, :, th * 512:(th + 1) * 512])
        wv_sb = sing.tile([P, NDT * P], bf16)
        nc.sync.dma_start(wv_sb, wv)
        wg_sb = sing.tile([P, NDT * 512], bf16)
        nc.scalar.dma_start(wg_sb, wg)
        wo_sb = sing.tile([P, NDT * 512], bf16)
        nc.gpsimd.dma_start(wo_sb, wo)
        xq_sb = sing.tile([P, NDT * 256], bf16)
        nc.sync.dma_start(xq_sb, xq)
        wnr_sb = sing.tile([1, 512], fp32)
        nc.sync.dma_start(wnr_sb, wnr)
        wnormF = sing.tile([P, 512], fp32)
        nc.gpsimd.partition_broadcast(wnormF, wnr_sb)
        wba_sb = sing.tile([P, NDT * 2], bf16)
        nc.sync.dma_start(wba_sb, wba)
        convw_sb = sing.tile([P, 36], fp32)
        nc.sync.dma_start(convw_sb, convw)
        sc_sb = sing.tile([1, 8], fp32)
        nc.sync.dma_start(sc_sb, sc)
        scB = sing.tile([P, 8], fp32)
        nc.gpsimd.partition_broadcast(scB, sc_sb)
        bm_sb = sing.tile([1, 2], fp32)
        nc.sync.dma_start(bm_sb, bmask)
        bmB = sing.tile([P, 2], fp32)
        nc.gpsimd.partition_broadcast(bmB, bm_sb)

        ident = sing.tile([P, P], fp32)
        make_identity(nc, ident)
        ones128 = sing.tile([P, P], fp32)
        nc.vector.memset(ones128, 1.0)
        epsc = sing.tile([P, 1], fp32)
        nc.vector.memset(epsc, 1e-6)
        epsDc = sing.tile([P, 1], fp32)
        nc.vector.memset(epsDc, float(D) * 1e-6)
        eps5c = sing.tile([P, 1], fp32)
        nc.vector.memset(eps5c, 1e-5)
        # one-hot row 127 selector
        oh127 = sing.tile([P, 1], fp32)
        nc.vector.memset(oh127, 1.0)
        nc.gpsimd.affine_select(oh127, oh127, [[0, 1]], OP.is_equal, 0.0,
                                base=-127, channel_multiplier=1)
        # lower-triangular (incl diag) ones, fp32: keep where f >= p
        trilf = sing.tile([P, P], fp32)
        nc.vector.memset(trilf, 1.0)
        nc.gpsimd.affine_select(trilf, trilf, [[1, P]], OP.is_ge, 0.0,
                                base=0, channel_multiplier=-1)

        xTp3 = xT_sb.rearrange("p (k t) -> p k t", k=NDT)  # t-block = 3+1024
        wqk3 = wqk_sb.rearrange("p (k t) -> p k t", k=NDT)
        wv3 = wv_sb.rearrange("p (k t) -> p k t", k=NDT)
        wba3 = wba_sb.rearrange("p (k t) -> p k t", k=NDT)
        wg3 = wg_sb.rearrange("p (k t) -> p k t", k=NDT)
        wo3 = wo_sb.rearrange("p (k t) -> p k t", k=NDT)
        xq3 = xq_sb.rearrange("p (k t) -> p k t", k=NDT)

        from concourse.tile import add_dep_helper as _adh
        act_order = []      # silu-table ops
        sqrt_late = []      # sqrt-table ops that must come after all silus

        # ── beta/a projections straight into column space [t-part, chunk] ──
        bac = sing.tile([P, NCH, 2], fp32)
        for c in range(NCH):
            bap = ps2.tile([P, 2], fp32, tag="pair", name="bap")
            for kt in range(NDT):
                nc.tensor.matmul(bap, xTp3[:, kt, 3 + c * P: 3 + (c + 1) * P],
                                 wba3[:, kt, :],
                                 start=(kt == 0), stop=(kt == NDT - 1))
            nc.vector.tensor_copy(bac[:, c], bap)
        bl_col = bac[:, :, 0]
        a_col = bac[:, :, 1]

        # g = -exp(A_log) * softplus(a + dt_bias) = ln(1+exp(a+bias)), col space
        onec = sing.tile([P, 1], fp32)
        nc.vector.memset(onec, 1.0)
        spe_col = sing.tile([P, NCH], fp32)
        nc.scalar.activation(spe_col, a_col, AF.Exp,
                             bias=scB[:, 0:1], scale=1.0)
        sp_col = sing.tile([P, NCH], fp32)
        nc.scalar.activation(sp_col, spe_col, AF.Ln, bias=onec[:, 0:1])
        g_col = sing.tile([P, NCH], fp32)
        nc.vector.tensor_scalar_mul(g_col, sp_col, scB[:, 1:2])
        # L = in-chunk inclusive cumsum of g  (tril ones matmul, fp32)
        Lp = ps2.tile([P, NCH], fp32, tag="pair", name="Lp")
        nc.tensor.matmul(Lp, trilf, g_col, start=True, stop=True)
        L_col = sing.tile([P, NCH], fp32)
        nc.vector.tensor_copy(L_col, Lp)

        # beta = sigmoid(bl) via exp + reciprocal (Exp table)
        bexp = sing.tile([P, NCH], fp32)
        nc.scalar.activation(bexp, bl_col, AF.Exp, scale=-1.0)
        bexp1 = sing.tile([P, NCH], fp32)
        nc.vector.tensor_scalar_add(bexp1, bexp, 1.0)
        bcol = sing.tile([P, NCH], fp32)
        nc.vector.reciprocal(bcol, bexp1)
        lam_col = sing.tile([P, NCH], fp32)
        nc.scalar.activation(lam_col, L_col, AF.Exp)
        eneg_col = sing.tile([P, NCH], fp32)
        nc.scalar.activation(eneg_col, L_col, AF.Exp, scale=-1.0)
        # lam per chunk end: lam8row = oh127^T @ lam_col, bcast down partitions
        l8p = ps2.tile([1, NCH], fp32, tag="pair", name="l8p")
        nc.tensor.matmul(l8p, oh127, lam_col, start=True, stop=True)
        lam8 = sing.tile([1, NCH], fp32)
        nc.vector.tensor_copy(lam8, l8p)
        lamB = sing.tile([P, NCH], fp32)
        nc.gpsimd.partition_broadcast(lamB, lam8)

        # conv-as-matmul: per (group, tap) diagonal weight tiles
        ident_bf = sing.tile([P, P], bf16)
        nc.vector.tensor_copy(ident_bf, ident)
        diag4 = [sing.tile([P, 4, P], bf16, tag=f"diag4_{g}", name=f"diag4_{g}")
                 for g in range(9)]
        for g in range(9):
            for j in range(4):
                nc.vector.tensor_scalar_mul(diag4[g][:, j, :], ident_bf,
                                            convw_sb[:, 4 * g + j:4 * g + j + 1])

        # ── projections into conv pads, then conv as diag matmuls + silu ──
        groups = ([("q", i) for i in range(NDT)] +
                  [("k", i) for i in range(NDT)] + [("v", 0)])
        pads = {}
        for name, dt_i in groups:
            pad = sing.tile([P, 3 + 1024], bf16, tag=f"pad_{name}{dt_i}",
                            name=f"pad_{name}{dt_i}")
            nc.vector.memset(pad[:, 0:3], 0.0)
            pads[(name, dt_i)] = pad

        ecnt = 0
        for th in range(2):  # t-half, 512 cols
            for name, dt_i in groups:
                pp = ps2.tile([P, 512], fp32, tag="big", name="pp", bufs=3)
                for kt in range(NDT):
                    if name == "q":
                        lhs_ = wqk3[:, kt, dt_i * P:(dt_i + 1) * P]
                    elif name == "k":
                        lhs_ = wqk3[:, kt, 512 + dt_i * P: 512 + (dt_i + 1) * P]
                    else:
                        lhs_ = wv3[:, kt]
                    nc.tensor.matmul(
                        pp, lhs_, xTp3[:, kt, 3 + th * 512: 3 + (th + 1) * 512],
                        start=(kt == 0), stop=(kt == NDT - 1))
                if ecnt % 2 == 0:
                    nc.scalar.copy(
                        pads[(name, dt_i)][:, 3 + th * 512: 3 + (th + 1) * 512], pp)
                else:
                    nc.vector.tensor_copy(
                        pads[(name, dt_i)][:, 3 + th * 512: 3 + (th + 1) * 512], pp)
                ecnt += 1

        # conv (4 taps) + silu (scalar act table).
        # kq_all[p, i, c, 0:128]=K-chunk, [...,128:256]=Q-chunk
        kq_all = sing.tile([P, NDT, NCH, 2 * P], bf16)
        kqcat = [kq_all[:, i] for i in range(NDT)]
        vTf = sing.tile([P, 1024], bf16)

        def kslice(i, c):
            return kq_all[:, i, c, 0:P]

        for th in range(2):
            for name, dt_i in groups:
                g = groups.index((name, dt_i))
                pad = pads[(name, dt_i)]
                cps = ps2.tile([P, 512], fp32, tag="big", name="cps", bufs=3)
                for j in range(4):
                    nc.tensor.matmul(cps, diag4[g][:, j],
                                     pad[:, th * 512 + j: th * 512 + j + 512],
                                     start=(j == 0), stop=(j == 3))
                cv = cps.rearrange("p (c t) -> p c t", c=4)
                crange = slice(th * 4, (th + 1) * 4)
                if USE_SILU_TABLE:
                    if name == "v":
                        act_order.append(nc.scalar.activation(
                            vTf[:, th * 512:(th + 1) * 512], cps, AF.Silu))
                    else:
                        off = 0 if name == "k" else P
                        act_order.append(nc.scalar.activation(
                            kqcat[dt_i][:, crange, off:off + P], cv, AF.Silu))
                else:
                    sg = sb2.tile([P, 512], bf16, tag="sgc", name="sgc")
                    nc.scalar.activation(sg, cps, AF.Sigmoid)
                    sgv = sg.rearrange("p (c t) -> p c t", c=4)
                    if name == "v":
                        nc.vector.tensor_mul(vTf[:, th * 512:(th + 1) * 512],
                                             cps, sg)
                    else:
                        off = 0 if name == "k" else P
                        nc.vector.tensor_mul(kqcat[dt_i][:, crange, off:off + P],
                                             cv, sgv)

        # ── gate projection + silu, early (only needs xq); tail uses wng ──
        wng2 = sing.tile([P, 2, 512], bf16)
        for tt in range(2):
            gp = ps2.tile([P, 512], fp32, tag="big", name="gp", bufs=3)
            for kt in range(NDT):
                nc.tensor.matmul(gp, xq3[:, kt, tt * P:(tt + 1) * P],
                                 wg3[:, kt, :],
                                 start=(kt == 0), stop=(kt == NDT - 1))
            gsil = sb2.tile([P, 512], fp32, tag="gsil", name="gsil")
            if USE_SILU_TABLE:
                act_order.append(nc.scalar.activation(gsil, gp, AF.Silu))
            else:
                gsg = sb2.tile([P, 512], fp32, tag="gsg", name="gsg")
                act_order.append(nc.scalar.activation(gsg, gp, AF.Sigmoid))
                nc.vector.tensor_mul(gsil, gp, gsg)
            nc.vector.tensor_mul(wng2[:, tt], gsil, wnormF)

        # ── chunk pass A: KK/KQ (stored) + QQ; diag -> sumsq cols ──
        kkq_sb = [sing.tile([P, 2 * P], bf16, tag=f"kkq{c}", name=f"kkq{c}")
                  for c in range(NCH)]
        ssqk_col = sing.tile([P, NCH], fp32)
        ssqq_col = sing.tile([P, NCH], fp32)
        for c in range(NCH):
            kkq = ps2.tile([P, 3 * P], fp32, tag="big", name="kkq", bufs=3)
            for i in range(NDT):
                nc.tensor.matmul(kkq[:, 0:2 * P], kq_all[:, i, c, 0:P],
                                 kq_all[:, i, c, :],
                                 start=(i == 0), stop=(i == NDT - 1))
            for i in range(NDT):
                nc.tensor.matmul(kkq[:, 2 * P:3 * P], kq_all[:, i, c, P:2 * P],
                                 kq_all[:, i, c, P:2 * P],
                                 start=(i == 0), stop=(i == NDT - 1))
            nc.vector.tensor_copy(kkq_sb[c], kkq[:, 0:2 * P])
            junk = sb2.tile([P, P], bf16, tag="junk", name="junk")
            nc.vector.scalar_tensor_tensor(junk, kkq_sb[c][:, 0:P], 1.0, ident,
                                           OP.mult, OP.mult,
                                           accum_out=ssqk_col[:, c:c + 1])
            junk2 = sb2.tile([P, P], bf16, tag="junk", name="junk2")
            nc.vector.scalar_tensor_tensor(junk2, kkq[:, 2 * P:3 * P], 1.0, ident,
                                           OP.mult, OP.mult,
                                           accum_out=ssqq_col[:, c:c + 1])

        # ── l2norm scales via Sqrt + reciprocal (Sqrt table, after silus) ──
        srk = sing.tile([P, NCH], fp32)
        sqrt_late.append(nc.scalar.activation(srk, ssqk_col, AF.Sqrt,
                                              bias=epsc[:, 0:1]))
        rk_col = sing.tile([P, NCH], fp32)
        nc.vector.reciprocal(rk_col, srk)
        srq = sing.tile([P, NCH], fp32)
        sqrt_late.append(nc.scalar.activation(srq, ssqq_col, AF.Sqrt,
                                              bias=epsDc[:, 0:1],
                                              scale=float(D)))
        rq_col = sing.tile([P, NCH], fp32)
        nc.vector.reciprocal(rq_col, srq)


        colf_col = sing.tile([P, NCH], fp32)
        nc.vector.tensor_mul(colf_col, eneg_col, rk_col)
        qf_col = sing.tile([P, NCH], fp32)
        nc.vector.tensor_mul(qf_col, lam_col, rq_col)
        rowfM_col = sing.tile([P, NCH], fp32)
        nc.vector.tensor_mul(rowfM_col, bcol, lam_col)
        nc.vector.tensor_mul(rowfM_col, rowfM_col, rk_col)
        rowfMn_col = sing.tile([P, NCH], fp32)
        nc.vector.tensor_scalar_mul(rowfMn_col, rowfM_col, -1.0)
        kbar_col = sing.tile([P, NCH], fp32)
        nc.vector.tensor_mul(kbar_col, colf_col, lamB)

        # colf as per-free chunk tiles: pack -> PE transpose -> broadcasts
        colf_bf = sing.tile([P, NCH], bf16)
        nc.vector.tensor_copy(colf_bf, colf_col)
        cfp = ps2.tile([NCH, P], bf16, tag="pair", name="cfp")
        nc.tensor.transpose(cfp, colf_bf, ident_bf)
        colfT = sing.tile([NCH, P], bf16)
        nc.vector.tensor_copy(colfT, cfp)
        colfDr = dramp.tile([NCH, P], bf16, name="colfDr")
        nc.sync.dma_start(colfDr, colfT)
        colfRow = sing.tile([1, NCH * P], bf16)
        nc.sync.dma_start(colfRow, colfDr.rearrange("c d -> (c d)"))
        colfBall = sing.tile([P, NCH * P], bf16)
        nc.gpsimd.partition_broadcast(colfBall, colfRow)
        colfB = [colfBall[:, c * P:(c + 1) * P] for c in range(NCH)]

        # ── V transpose + beta scale ──
        Vb = [sing.tile([P, P], bf16, tag=f"Vb{c}", name=f"Vb{c}")
              for c in range(NCH)]
        for c in range(NCH):
            tp = ps2.tile([P, 2 * P], bf16, tag="pair", name="vtp")[:, 0:P]
            nc.tensor.transpose(tp, vTf[:, c * C:(c + 1) * C], ident_bf)
            nc.vector.tensor_scalar_mul(Vb[c], tp, bcol[:, c:c + 1])

        # ── K natural-layout [t, d] chunks, pre-scaled by kbar ──
        KnS = [sing.tile([P, D], bf16, tag=f"KnS{c}", name=f"KnS{c}")
               for c in range(NCH)]
        for c in range(NCH):
            for half in range(2):
                tpk = ps2.tile([P, 2 * P], bf16, tag="pair", name="tpk")
                nc.tensor.transpose(tpk[:, 0:P], kslice(2 * half, c), ident_bf)
                nc.tensor.transpose(tpk[:, P:2 * P], kslice(2 * half + 1, c), ident_bf)
                nc.vector.tensor_scalar_mul(
                    KnS[c][:, half * 2 * P:(half + 1) * 2 * P], tpk,
                    kbar_col[:, c:c + 1])

        # ── chunk pass B: M, MT, M2, TT, AmatT ──
        AmatT = [sing.tile([P, P], bf16, tag=f"Am{c}", name=f"Am{c}")
                 for c in range(NCH)]
        TTs = [sing.tile([P, P], bf16, tag=f"TT{c}", name=f"TT{c}")
               for c in range(NCH)]
        for c in range(NCH):
            # M [t,i] strict lower
            M = sb2.tile([P, P], bf16, tag="M", name="M", bufs=3)
            nc.vector.scalar_tensor_tensor(M, kkq_sb[c][:, 0:P],
                                           rowfM_col[:, c:c + 1],
                                           colfB[c], OP.mult, OP.mult)
            nc.gpsimd.affine_select(M, M, [[-1, P]], OP.is_ge, 0.0,
                                    base=-1, channel_multiplier=1)
            # MT via PE transpose (already masked)
            mtp = ps2.tile([P, 2 * P], bf16, tag="pair", name="cm")[:, 0:P]
            nc.tensor.transpose(mtp, M, ident_bf)
            MT = sb2.tile([P, P], bf16, tag="MT", name="MT", bufs=3)
            nc.vector.tensor_copy(MT, mtp)
            # AmatT [i,t] upper incl diag
            nc.vector.tensor_scalar_mul(AmatT[c], kkq_sb[c][:, P:2 * P],
                                        colf_col[:, c:c + 1])
            nc.gpsimd.affine_select(AmatT[c], AmatT[c], [[1, P]], OP.is_ge,
                                    0.0, base=0, channel_multiplier=-1)
            # M2
            pr = ps2.tile([P, 2 * P], fp32, tag="pair", name="pair")
            nc.tensor.matmul(pr[:, 0:P], MT, M, start=True, stop=True)
            pw0 = sb2.tile([P, P], bf16, tag="pw0", name="pw0", bufs=3)
            nc.vector.tensor_copy(pw0, pr[:, 0:P])
            # TT = (I + M2T)(I - MT)
            tt = sb2.tile([P, P], bf16, tag="ttp", name="ttp", bufs=3)
            nc.vector.tensor_sub(tt, ident, MT)
            tp_ = ps2.tile([P, P], fp32, tag="pair", name="ttps")
            nc.tensor.matmul(tp_, pw0, tt, start=True, stop=True)
            nc.vector.tensor_add(TTs[c], tt, tp_)

        # ── serial scan; o chunks stream to DRAM for the AllToAll ──
        # 8-core AllToAll per t-half: block j of call h carries my o chunk
        # (4h + j%4); each core r=(b,s) receives, in row-block i, core i's
        # chunk (4h+s) = global t rows [512h+128s : +128] for dv-slice i%4.
        # Cross-batch blocks (i//4 != b) are discarded with the bmask merge.
        S_t = sing.tile([P, NDT, P], bf16, tag="St_1", name="St_init")
        nc.vector.memset(S_t, 0.0)
        a2a_in = dramp.tile([8, 2, P, P], bf16, name="a2a_in")
        a2a_out = dramp.tile([8, 2, P, P], bf16, name="a2a_out")

        for c in range(NCH):
            ksp = ps.tile([P, P], fp32, tag="h", name="ksp", bufs=3)
            op_ = ps.tile([P, P], fp32, tag="h", name="op_", bufs=3)
            for i in range(NDT):
                nc.tensor.matmul(ksp, kq_all[:, i, c, 0:P], S_t[:, i],
                                 start=(i == 0), stop=(i == NDT - 1))
            # o partial: sum_d QT.T S (fills PE while rhsw is built)
            for i in range(NDT):
                nc.tensor.matmul(op_, kq_all[:, i, c, P:2 * P], S_t[:, i],
                                 start=(i == 0), stop=False)
            rhsw = sb2.tile([P, P], bf16, tag="rhsw", name="rhsw", bufs=3)
            nc.vector.scalar_tensor_tensor(rhsw, ksp, rowfMn_col[:, c:c + 1],
                                           Vb[c], OP.mult, OP.add)
            wp = ps2.tile([P, P], fp32, tag="pair", name="wp")
            nc.tensor.matmul(wp, TTs[c], rhsw, start=True, stop=True)
            W = sb2.tile([P, P], bf16, tag="W", name="W", bufs=3)
            nc.vector.tensor_copy(W, wp)
            # S update: S[i] = lam*S[i] + KnS[i].T @ W
            sup = ps2.tile([P, 4 * P], fp32, tag="big", name="sup", bufs=3)
            for i in range(NDT):
                nc.tensor.matmul(sup[:, i * P:(i + 1) * P],
                                 KnS[c][:, i * P:(i + 1) * P], W,
                                 start=True, stop=True)
            nc.tensor.matmul(op_, AmatT[c], W, start=False, stop=True)
            o_sb = sb2.tile([P, P], bf16, tag="osb", name="osb", bufs=3)
            nc.scalar.mul(o_sb, op_, qf_col[:, c:c + 1])
            h, j = c // 4, c % 4
            nc.sync.dma_start(a2a_in[j, h], o_sb)
            nc.sync.dma_start(a2a_in[j + 4, h], o_sb)
            newS = sing.tile([P, NDT, P], bf16, tag=f"St_{c % 2}",
                             name=f"St_{c % 2}")
            nSf = newS.rearrange("p i t -> p (i t)")
            oSf = S_t.rearrange("p i t -> p (i t)")
            for hh in range(2):
                nc.vector.scalar_tensor_tensor(
                    nSf[:, hh * 2 * P:(hh + 1) * 2 * P],
                    oSf[:, hh * 2 * P:(hh + 1) * 2 * P], lamB[:, c:c + 1],
                    sup[:, hh * 2 * P:(hh + 1) * 2 * P], OP.mult, OP.add)
            S_t = newS
            if c == 7:
                nc.gpsimd.collective_compute(
                    kind="AllToAll", op=OP.bypass,
                    replica_groups=[[0, 1, 2, 3, 4, 5, 6, 7]],
                    ins=[a2a_in[:]], outs=[a2a_out[:]])

        # ── tail: RMSNorm + silu(gate) + output projection, per t-tile ──
        for tt in range(2):
            a2asb = sing.tile([P, 8, P], bf16, tag=f"a2asb{tt}",
                              name=f"a2asb{tt}")
            nc.sync.dma_start(
                a2asb, a2a_out.rearrange("i h p d -> h p i d")[tt])
            # merge batch halves: oq = blk[b0]*m0 + blk[b1]*m1
            oq = sing.tile([P, 4, P], bf16, tag=f"oq{tt}", name=f"oq{tt}")
            tmpo = sb2.tile([P, 4, P], bf16, tag="tmpo", name="tmpo")
            nc.vector.tensor_scalar_mul(tmpo, a2asb[:, 4:8, :], bmB[:, 1:2])
            nc.vector.scalar_tensor_tensor(oq, a2asb[:, 0:4, :], bmB[:, 0:1],
                                           tmpo, OP.mult, OP.add)
            oqf = oq.rearrange("p i d -> p (i d)")
            junko = sb2.tile([P, 512], bf16, tag="junko", name="junko")
            ssq2 = sing.tile([P, 1], fp32, tag=f"ssq{tt}", name=f"ssq{tt}")
            nc.scalar.activation(junko, oqf, AF.Square,
                                 accum_out=ssq2[:, 0:1])
            srt = sing.tile([P, 1], fp32, tag=f"srt{tt}", name=f"srt{tt}")
            nc.scalar.activation(srt, ssq2, AF.Sqrt, bias=eps5c[:, 0:1],
                                 scale=1.0 / D)
            rsq = sing.tile([P, 1], fp32, tag=f"rsq{tt}", name=f"rsq{tt}")
            nc.vector.reciprocal(rsq, srt)
            t3 = sb2.tile([P, 512], bf16, tag="t3", name="t3")
            nc.vector.scalar_tensor_tensor(t3, oqf, rsq[:, 0:1], wng2[:, tt],
                                           OP.mult, OP.mult)
            # transpose t3 -> [d, t] tiles
            t3T = sing.tile([P, NDT, P], bf16, tag=f"t3T{tt}", name=f"t3T{tt}")
            for h2 in range(2):
                tpt = ps2.tile([P, 2 * P], bf16, tag="pair", name="tpt")
                nc.tensor.transpose(tpt[:, 0:P], t3[:, (2 * h2) * P:(2 * h2 + 1) * P],
                                    ident_bf)
                nc.tensor.transpose(tpt[:, P:2 * P],
                                    t3[:, (2 * h2 + 1) * P:(2 * h2 + 2) * P],
                                    ident_bf)
                nc.vector.tensor_copy(
                    t3T.rearrange("p i d -> p (i d)")[:, h2 * 2 * P:(h2 + 1) * 2 * P],
                    tpt)
            # output projection: accumulate over d tiles
            po = ps2.tile([P, 512], fp32, tag="big", name="po", bufs=3)
            for i in range(NDT):
                nc.tensor.matmul(po, t3T[:, i], wo3[:, i, :],
                                 start=(i == 0), stop=(i == NDT - 1))
            ob = sb2.tile([P, 512], fp32, tag="ob", name="ob")
            nc.vector.tensor_copy(ob, po)
            nc.sync.dma_start(out_rows[tt * P:(tt + 1) * P, :], ob)


# ───────────────────────────── host-side prep ─────────────────────────────
def _tile512(a):
    # [512, N] -> [128, 4*N] with col = kt*N + j
    n = a.shape[1]
    return np.ascontiguousarray(
        a.reshape(NDT, P, n).transpose(1, 0, 2).reshape(P, NDT * n))


def prep_l1(x, q_proj_w, k_proj_w, v_proj_w, b_proj_w, a_proj_w, A_log,
            dt_bias, q_conv_w, k_conv_w, v_conv_w, g_proj_w=None,
            o_norm_w=None, o_proj_w=None):
    sc = np.zeros((1, 8), np.float32)
    sc[0, 0] = float(dt_bias[0])
    sc[0, 1] = -float(np.exp(A_log[0]))
    # padded xT: per kt block [128, 3+1024]
    xTs = []
    for b in range(B):
        xb = x[b].T.reshape(NDT, P, 1024)
        xp = np.concatenate([np.zeros((NDT, P, 3), np.float32), xb], 2)
        xTs.append(np.ascontiguousarray(
            xp.transpose(1, 0, 2).reshape(P, NDT * 1027)).astype(BF16))
    wgT = _tile512(np.ascontiguousarray(g_proj_w.T)).astype(BF16)
    woT = _tile512(np.ascontiguousarray(o_proj_w.T)).astype(BF16)
    wnr = np.ascontiguousarray(o_norm_w).reshape(1, 512).astype(np.float32)
    wba = _tile512(np.concatenate([b_proj_w.T, a_proj_w.T], 1)).astype(BF16)
    wqk = _tile512(np.concatenate([q_proj_w.T, k_proj_w.T], 1)).astype(BF16)
    ins = []
    for b in range(B):
        for s in range(NDT):
            vsl = slice(s * P, (s + 1) * P)
            convw = np.zeros((P, 36), np.float32)
            for i in range(NDT):
                convw[:, 4 * i:4 * (i + 1)] = q_conv_w[i * P:(i + 1) * P]
                convw[:, 16 + 4 * i:16 + 4 * (i + 1)] = k_conv_w[i * P:(i + 1) * P]
            convw[:, 32:36] = v_conv_w[vsl]
            xb = x[b].T
            xqc = np.concatenate([xb[:, s * P:(s + 1) * P],
                                  xb[:, 512 + s * P:512 + (s + 1) * P]], 1)
            bm = np.zeros((1, 2), np.float32)
            bm[0, b] = 1.0
            ins.append({
                "xT": xTs[b],
                "wqk": wqk,
                "wv": _tile512(np.ascontiguousarray(v_proj_w.T[:, vsl])).astype(BF16),
                "wba": wba,
                "convw": convw,
                "sc": sc,
                "wg": wgT,
                "wo": woT,
                "wnr": wnr,
                "xq": _tile512(np.ascontiguousarray(xqc)).astype(BF16),
                "bmask": bm,
            })
    return ins


# ─────────────────────────── build + run (spmd) ───────────────────────────
def _build(kern, in_specs, out_specs):
    import concourse.mybir as mybir
    import concourse.tile as tile
    from concourse import bacc
    nc = bacc.Bacc(None, target_bir_lowering=False)
    with tile.TileContext(nc) as tc:
        with tc.tile_pool(name="io", bufs=1, space="DRAM") as io:
            ins = {k: io.tile(shape, dt, kind="ExternalInput", name=f"in_{k}")
                   for k, (shape, dt) in in_specs.items()}
            outs = {k: io.tile(shape, dt, kind="ExternalOutput", name=f"out_{k}")
                    for k, (shape, dt) in out_specs.items()}
            kern(tc, {k: v[:] for k, v in ins.items()},
                 {k: v[:] for k, v in outs.items()})
    nc.compile()
    return nc, ins, outs


_CACHE = {}


def _specs_l1():
    import concourse.mybir as mybir
    f, h = mybir.dt.float32, mybir.dt.bfloat16
    in_specs = {"xT": ((P, NDT * 1027), h), "wqk": ((P, NDT * 1024), h),
                "wv": ((P, NDT * P), h),
                "convw": ((P, 36), f),
                "wba": ((P, NDT * 2), h), "sc": ((1, 8), f),
                "wg": ((P, NDT * 512), h), "wo": ((P, NDT * 512), h),
                "wnr": ((1, 512), f), "xq": ((P, NDT * 256), h),
                "bmask": ((1, 2), f)}
    out_specs = {"out": ((2 * P, 512), f)}
    return in_specs, out_specs


def run_spmd(which, kern, specs, in_dicts, trace):
    from concourse.bass_utils import run_bass_kernel_spmd
    install_ntff_shim()
    if which not in _CACHE:
        _CACHE[which] = _build(kern, *specs)
    nc, ins, outs = _CACHE[which]
    in_maps = [{ins[k].name: np.ascontiguousarray(v) for k, v in d.items()}
               for d in in_dicts]
    t0 = time.perf_counter()
    try:
        res = run_bass_kernel_spmd(nc, in_maps, list(range(len(in_dicts))),
                                   trace=trace)
    except Exception:
        if not trace:
            raise
        res = run_bass_kernel_spmd(nc, in_maps, list(range(len(in_dicts))),
                                   trace=False)
    wall_ns = int((time.perf_counter() - t0) * 1e9)
    outl = [{k: np.asarray(res.results[c][outs[k].name])
             for k in outs} for c in range(len(in_dicts))]
    return outl, (res.exec_time_ns if res.exec_time_ns else wall_ns)


def kernel(x, q_proj_w, k_proj_w, v_proj_w, b_proj_w, a_proj_w, A_log,
           dt_bias, q_conv_w, k_conv_w, v_conv_w, g_proj_w, o_norm_w,
           o_proj_w, trace=True):
    args = [np.asarray(a, np.float32) for a in
            (x, q_proj_w, k_proj_w, v_proj_w, b_proj_w, a_proj_w, A_log,
             dt_bias, q_conv_w, k_conv_w, v_conv_w, g_proj_w, o_norm_w,
             o_proj_w)]
    (x, q_proj_w, k_proj_w, v_proj_w, b_proj_w, a_proj_w, A_log, dt_bias,
     q_conv_w, k_conv_w, v_conv_w, g_proj_w, o_norm_w, o_proj_w) = args

    ins1 = prep_l1(x, q_proj_w, k_proj_w, v_proj_w, b_proj_w, a_proj_w,
                   A_log, dt_bias, q_conv_w, k_conv_w, v_conv_w,
                   g_proj_w, o_norm_w, o_proj_w)
    r1, ns1 = run_spmd("l1", l1_kernel, _specs_l1(), ins1, trace)
    out = np.zeros((B, T, D), np.float32)
    for b in range(B):
        for s in range(NDT):
            r = r1[b * 4 + s]["out"].astype(np.float32)
            out[b, s * P:(s + 1) * P] = r[0:P]
            out[b, 512 + s * P:512 + (s + 1) * P] = r[P:2 * P]
    _LAST_HW_NS[0] = ns1
    return out
